# revision 1
# baseline (speedup 1.0000x reference)
"""GQA attention block (wq/wk/wv -> RoPE -> attention -> wo) on 8 TRN2 cores.

Sharding: tensor-parallel over heads. Core j owns kv-head j and q-heads
{j, j+8, j+16, j+24}. Each core computes a full [S, E] partial of the output
projection (contraction over its 256 head-dims of wo); partials are summed on
the host.

Layout strategy (per core; matmul operands bf16, accumulation fp32):
  - Activations are fed host-transposed and host-cast: xT [E, S] bf16.
    Projections produce QT/KT [d, S] directly (d on partitions), so scores
    come out transposed S^T [k, q], which is exactly the orientation the AV
    matmul needs as its moving operand.
  - Head-dim rows of wq/wk are host-permuted to [evens, odds] so RoPE becomes
    a 32-partition block swap (SBUF-to-SBUF DMA) + 3 full-width DVE ops. The
    permutation cancels in the q.k contraction.
  - Softmax denominator rides the AV matmul as a 65th column of ones in V.
    No max-subtraction pass (scores are O(+-6); exp is safe).
  - attention_mask folded in as a per-partition bias on the exp activation.
"""

import sys

sys.path.insert(0, "/opt/trn_rl_repo")

from contextlib import ExitStack

import ml_dtypes
import numpy as np

import concourse.bacc as bacc
import concourse.bass as bass
import concourse.tile as tile
from concourse import mybir
from concourse.bass_utils import run_bass_kernel_spmd

P = 128
S = 2048  # sequence length
E = 2048  # embed dim
D = 64    # head dim
EK = E // P   # 16 contraction tiles for projections
SK = S // P   # 16 key tiles for attention
NCORES = 8
QH = 2        # attention processed in QH slices of the query dim
QW = S // QH  # 1024
F32 = mybir.dt.float32
BF16 = mybir.dt.bfloat16
AF = mybir.ActivationFunctionType
BF16NP = ml_dtypes.bfloat16


def build_bass(repeat=1):
    nc = bacc.Bacc()
    xqT = nc.declare_dram_parameter("xqT", [E, S], BF16, isOutput=False)
    xkvT = nc.declare_dram_parameter("xkvT", [E, S], BF16, isOutput=False)
    wqT = nc.declare_dram_parameter("wqT", [E, 256], BF16, isOutput=False)
    wkvT = nc.declare_dram_parameter("wkvT", [E, P], BF16, isOutput=False)
    woT = nc.declare_dram_parameter("woT", [256, E], BF16, isOutput=False)
    rqc = nc.declare_dram_parameter("rqc", [P, S], BF16, isOutput=False)
    rqs = nc.declare_dram_parameter("rqs", [P, S], BF16, isOutput=False)
    rkc = nc.declare_dram_parameter("rkc", [P, S], BF16, isOutput=False)
    rks = nc.declare_dram_parameter("rks", [P, S], BF16, isOutput=False)
    mbias = nc.declare_dram_parameter("mbias", [P, SK], F32, isOutput=False)
    ident = nc.declare_dram_parameter("ident", [P, P], BF16, isOutput=False)
    outp = nc.declare_dram_parameter("out_partial", [S, E], F32, isOutput=True)

    with ExitStack() as ctx:
        tc = ctx.enter_context(tile.TileContext(nc))
        persist = ctx.enter_context(tc.tile_pool(name="persist", bufs=1))

        qt0 = persist.tile([P, S], BF16, tag="qt0")
        qt1 = persist.tile([P, S], BF16, tag="qt1")
        qt = [qt0, qt1]
        ktdup = persist.tile([P, S], BF16, tag="ktdup")
        v_sb = persist.tile([P, SK, 65], BF16, tag="v_sb")  # V natural + ones col
        kv_sb = persist.tile([P, S], BF16, tag="kv_sb")  # KT(0:64)+VT(64:128)
        oA = persist.tile([P, S], BF16, tag="oA")
        oB = persist.tile([P, S], BF16, tag="oB")
        mb_sb = persist.tile([P, SK], F32, tag="mb_sb")
        id_sb = persist.tile([P, P], BF16, tag="id_sb")
        # weights resident whole-kernel, single DMA each
        wq_sb = persist.tile([P, EK, 256], BF16, tag="wq_sb")
        wkv_sb = persist.tile([P, EK, P], BF16, tag="wkv_sb")
        wo_sb = persist.tile([P, 2, S], BF16, tag="wo_sb")
        rq_c = persist.tile([P, S], BF16, tag="rq_c")
        rq_s = persist.tile([P, S], BF16, tag="rq_s")
        rk_c = persist.tile([P, S], BF16, tag="rk_c")
        rk_s = persist.tile([P, S], BF16, tag="rk_s")
        ones_sb = persist.tile([1, P], BF16, tag="ones_sb")
        nc.vector.memset(ones_sb[:], 1.0)

        wq_r = wqT.ap().rearrange("(k p) c -> p k c", p=P)
        nc.sync.dma_start(out=wq_sb[:, 0:4, :], in_=wq_r[:, 0:4, :])
        nc.sync.dma_start(out=wq_sb[:, 4:EK, :], in_=wq_r[:, 4:EK, :])
        nc.scalar.dma_start(
            out=wkv_sb[:], in_=wkvT.ap().rearrange("(k p) c -> p k c", p=P)
        )
        nc.scalar.dma_start(out=mb_sb[:], in_=mbias[:])
        nc.scalar.dma_start(out=id_sb[:], in_=ident[:])
        nc.scalar.dma_start(out=rq_c[:], in_=rqc[:])
        nc.scalar.dma_start(out=rq_s[:], in_=rqs[:])
        nc.scalar.dma_start(out=rk_c[:], in_=rkc[:])
        nc.scalar.dma_start(out=rk_s[:], in_=rks[:])
        nc.scalar.dma_start(
            out=wo_sb[:], in_=woT.ap().rearrange("(k p) c -> p k c", p=P)
        )

        ppool = ctx.enter_context(tc.tile_pool(name="ppool", bufs=4, space="PSUM"))
        upool = ctx.enter_context(tc.tile_pool(name="upool", bufs=3))
        xpool = ctx.enter_context(tc.tile_pool(name="xpool", bufs=3))
        swpool = ctx.enter_context(tc.tile_pool(name="swpool", bufs=2))
        epool = ctx.enter_context(tc.tile_pool(name="epool", bufs=6))
        rcpool = ctx.enter_context(tc.tile_pool(name="rcpool", bufs=2))
        otpool = ctx.enter_context(tc.tile_pool(name="otpool", bufs=2))
        ostage = ctx.enter_context(tc.tile_pool(name="ostage", bufs=3))

        for rep in range(repeat):
            # ================= Phase 1: projections =================
            # Q projection: QT packs [128, S]; pack p holds q-heads 2p, 2p+1.
            # x tiles loaded 2 E-ktiles per DMA.
            qps = [
                [
                    ppool.tile([P, 1024], F32, tag="pb", name=f"r{rep}_qps{p_}_{c2}")
                    for c2 in range(2)
                ]
                for p_ in range(2)
            ]
            for k2 in range(EK // 2):
                xt = xpool.tile([P, 2, S], BF16, tag="x", name=f"r{rep}_xq{k2}")
                nc.sync.dma_start(
                    out=xt[:],
                    in_=xqT[k2 * 2 * P:(k2 + 1) * 2 * P, :].rearrange(
                        "(b p) s -> p b s", p=P
                    ),
                )
                for b in range(2):
                    kt = k2 * 2 + b
                    for p_ in range(2):
                        for c in range(4):
                            nc.tensor.matmul(
                                qps[p_][c // 2][:, (c % 2) * 512:(c % 2) * 512 + 512],
                                wq_sb[:, kt, p_ * P:(p_ + 1) * P],
                                xt[:, b, c * 512:(c + 1) * 512],
                                start=(kt == 0),
                                stop=(kt == EK - 1),
                            )
            for p_ in range(2):
                for c2 in range(2):
                    nc.vector.tensor_copy(
                        qt[p_][:, c2 * 1024:(c2 + 1) * 1024], qps[p_][c2][:]
                    )

            # K/V projection packed: one matmul per (kt, c); out rows 0:64 = KT,
            # rows 64:128 = VT
            kvps = [
                ppool.tile([P, 1024], F32, tag="pb", name=f"r{rep}_kvps{c2}")
                for c2 in range(2)
            ]
            for k2 in range(EK // 2):
                xt = xpool.tile([P, 2, S], BF16, tag="x", name=f"r{rep}_xkv{k2}")
                nc.sync.dma_start(
                    out=xt[:],
                    in_=xkvT[k2 * 2 * P:(k2 + 1) * 2 * P, :].rearrange(
                        "(b p) s -> p b s", p=P
                    ),
                )
                for b in range(2):
                    kt = k2 * 2 + b
                    for c in range(4):
                        nc.tensor.matmul(
                            kvps[c // 2][:, (c % 2) * 512:(c % 2) * 512 + 512],
                            wkv_sb[:, kt, :],
                            xt[:, b, c * 512:(c + 1) * 512],
                            start=(kt == 0),
                            stop=(kt == EK - 1),
                        )
            for c2 in range(2):
                nc.vector.tensor_copy(
                    kv_sb[:, c2 * 1024:(c2 + 1) * 1024], kvps[c2][:]
                )

            # V to natural layout [k, d] via PE transpose; ones col via one memset
            for sk in range(SK):
                tp = ppool.tile([P, D], BF16, tag="pb", name=f"r{rep}_vtp{sk}")
                nc.tensor.transpose(
                    tp[:, :],
                    kv_sb[D:P, sk * P:(sk + 1) * P],
                    id_sb[D:P, D:P],
                )
                nc.vector.tensor_copy(v_sb[:, sk, 0:D], tp[:, :])
            nc.vector.memset(v_sb[:, :, D:65], 1.0)

            # ================= Phase 2: RoPE =================
            for i, t in enumerate([qt0, qt1]):
                sw = swpool.tile([P, S], BF16, tag="sw", name=f"r{rep}_sw{i}")
                for blk in range(4):
                    sb = blk ^ 1
                    nc.sync.dma_start(
                        out=sw[blk * 32:(blk + 1) * 32, :],
                        in_=t[sb * 32:(sb + 1) * 32, :],
                    )
                nc.vector.tensor_mul(t[:], t[:], rq_c[:])
                nc.vector.tensor_mul(sw[:], sw[:], rq_s[:])
                nc.vector.tensor_add(t[:], t[:], sw[:])

            # K rope on kv_sb[0:64] -> ktdup[0:64], then duplicate to [64:128]
            swk = swpool.tile([P, S], BF16, tag="sw", name=f"r{rep}_swk")
            nc.sync.dma_start(out=swk[0:32, :], in_=kv_sb[32:64, :])
            nc.sync.dma_start(out=swk[32:64, :], in_=kv_sb[0:32, :])
            nc.vector.tensor_mul(ktdup[0:D, :], kv_sb[0:D, :], rk_c[0:D, :])
            nc.vector.tensor_mul(swk[0:D, :], swk[0:D, :], rk_s[0:D, :])
            nc.vector.tensor_add(ktdup[0:D, :], ktdup[0:D, :], swk[0:D, :])
            nc.sync.dma_start(out=ktdup[D:P, :], in_=ktdup[0:D, :])

            # ================= Phase 3: attention =================
            for qh in range(QH):
                for pk in range(2):
                    ups = [
                        ppool.tile([P, QW], F32, tag="pb", name=f"r{rep}_u{qh}{pk}{hh}")
                        for hh in range(2)
                    ]
                    for kt in range(SK):
                        # both heads' score matmuls emitted back-to-back: they
                        # target disjoint PE row groups (0:63 / 64:127) and can
                        # overlap in the array
                        sps, ets = [], []
                        for hh in range(2):
                            sp = ppool.tile([P, QW], F32, tag="pb", name=f"r{rep}_sp{hh}")
                            sps.append(sp)
                        for c in range(2):
                            for hh in range(2):
                                nc.tensor.matmul(
                                    sps[hh][:, c * 512:(c + 1) * 512],
                                    ktdup[
                                        hh * D:(hh + 1) * D,
                                        kt * P:(kt + 1) * P,
                                    ],
                                    qt[pk][
                                        hh * D:(hh + 1) * D,
                                        qh * QW + c * 512:
                                        qh * QW + (c + 1) * 512,
                                    ],
                                    start=True,
                                    stop=True,
                                )
                        for hh in range(2):
                            et = epool.tile([P, QW], BF16, tag="e", name=f"r{rep}_e{hh}")
                            nc.scalar.activation(
                                et[:],
                                sps[hh][:],
                                AF.Exp,
                                bias=mb_sb[:, kt:kt + 1],
                                scale=0.125,
                            )
                            ets.append(et)
                        for hh in range(2):
                            for c in range(2):
                                nc.tensor.matmul(
                                    ups[hh][0:65, c * 512:(c + 1) * 512],
                                    v_sb[:, kt, :],
                                    ets[hh][:, c * 512:(c + 1) * 512],
                                    start=(kt == 0),
                                    stop=(kt == SK - 1),
                                )

                    # drain: evict U to SBUF (frees PSUM), then divide by the
                    # ones-column sums and place into the O packs
                    for hh in range(2):
                        i = pk * 2 + hh
                        u_sb = upool.tile([65, QW], F32, tag="u_sb", name=f"r{rep}_us{i}")
                        nc.vector.tensor_copy(u_sb[:], ups[hh][0:65, :])
                        rc = rcpool.tile([1, QW], BF16, tag="rc", name=f"r{rep}_rc{i}")
                        with nc.allow_low_precision(
                            reason="softmax denom recip at bf16 matches bf16 pipeline"
                        ):
                            nc.vector.reciprocal(rc[0:1, :], u_sb[D:D + 1, :])
                        # broadcast recip across partitions with a K=1 matmul
                        bt = ppool.tile([P, QW], F32, tag="pb", name=f"r{rep}_b{i}")
                        for c in range(2):
                            nc.tensor.matmul(
                                bt[:, c * 512:(c + 1) * 512],
                                ones_sb[:],
                                rc[0:1, c * 512:(c + 1) * 512],
                                start=True,
                                stop=True,
                            )
                        dest = oA if (i % 2 == 0) else oB
                        base = (i // 2) * D
                        if base == 0:
                            nc.vector.tensor_mul(
                                dest[0:D, qh * QW:(qh + 1) * QW],
                                u_sb[0:D, :],
                                bt[0:D, :],
                            )
                        else:
                            ot = otpool.tile([D, QW], BF16, tag="ot", name=f"r{rep}_ot{i}")
                            nc.vector.tensor_mul(ot[:], u_sb[0:D, :], bt[0:D, :])
                            nc.sync.dma_start(
                                out=dest[base:base + D, qh * QW:(qh + 1) * QW],
                                in_=ot[:],
                            )

            # ================= Phase 4: output projection =================
            for m2 in range(SK // 2):
                ost = ostage.tile([P, 2, S], F32, tag="ost", name=f"r{rep}_ost{m2}")
                for b in range(2):
                    ms = m2 * 2 + b
                    for c2 in range(2):
                        wp = ppool.tile([P, 1024], F32, tag="pb", name=f"r{rep}_wp{c2}")
                        for h in range(2):
                            c = c2 * 2 + h
                            nc.tensor.matmul(
                                wp[:, h * 512:(h + 1) * 512],
                                oA[:, ms * P:(ms + 1) * P],
                                wo_sb[:, 0, c * 512:(c + 1) * 512],
                                start=True,
                                stop=False,
                            )
                            nc.tensor.matmul(
                                wp[:, h * 512:(h + 1) * 512],
                                oB[:, ms * P:(ms + 1) * P],
                                wo_sb[:, 1, c * 512:(c + 1) * 512],
                                start=False,
                                stop=True,
                            )
                        nc.vector.tensor_copy(
                            ost[:, b, c2 * 1024:(c2 + 1) * 1024], wp[:]
                        )
                nc.sync.dma_start(
                    out=outp[m2 * 2 * P:(m2 + 1) * 2 * P, :].rearrange(
                        "(b p) s -> p b s", p=P
                    ),
                    in_=ost[:],
                )

    nc.compile()
    return nc


_PERM = np.concatenate([np.arange(0, D, 2), np.arange(1, D, 2)])


def _host_inputs(inputs):
    """Build the shared and per-core device input maps."""
    q = np.asarray(inputs["query_states"], np.float32)[0].T.astype(BF16NP)
    kv = np.asarray(inputs["key_value_states"], np.float32)[0].T.astype(BF16NP)
    wq = np.asarray(inputs["wq"], np.float32)
    wk = np.asarray(inputs["wk"], np.float32)
    wv = np.asarray(inputs["wv"], np.float32)
    wo = np.asarray(inputs["wo"], np.float32)
    cos_q = np.asarray(inputs["cos_q"], np.float32)
    sin_q = np.asarray(inputs["sin_q"], np.float32)
    cos_k = np.asarray(inputs["cos_k"], np.float32)
    sin_k = np.asarray(inputs["sin_k"], np.float32)
    mask = np.asarray(inputs["attention_mask"]).reshape(S)

    def rope_arrays(cos, sin):
        ct = np.ascontiguousarray(cos.T)  # [32, S]
        st = np.ascontiguousarray(sin.T)
        rc = np.tile(ct, (4, 1))
        rs = np.tile(np.concatenate([-st, st], axis=0), (2, 1))
        return (
            np.ascontiguousarray(rc.astype(BF16NP)),
            np.ascontiguousarray(rs.astype(BF16NP)),
        )

    rq_c, rq_s = rope_arrays(cos_q, sin_q)
    rk_c, rk_s = rope_arrays(cos_k, sin_k)
    mb = np.where(mask, 0.0, -30000.0).astype(np.float32)
    mb = np.ascontiguousarray(mb.reshape(SK, P).T)  # [P, SK]
    ident = np.eye(P, dtype=BF16NP)

    shared = {
        "xqT": np.ascontiguousarray(q),
        "xkvT": np.ascontiguousarray(kv),
        "rqc": rq_c,
        "rqs": rq_s,
        "rkc": rk_c,
        "rks": rk_s,
        "mbias": mb,
        "ident": ident,
    }

    in_maps = []
    for j in range(NCORES):
        heads = [j, j + 8, j + 16, j + 24]
        wqTh = np.empty((E, 256), np.float32)
        for i, h in enumerate(heads):
            wqTh[:, i * D:(i + 1) * D] = wq[h * D + _PERM, :].T
        wk_p = wk[j * D + _PERM, :].T  # [E, 64]
        wv_p = wv[j * D:(j + 1) * D, :].T  # [E, 64]
        wkvTh = np.concatenate([wk_p, wv_p], axis=1)
        # O packs: oA rows = heads (0, 2), oB rows = heads (1, 3)
        woTh = np.empty((256, E), np.float32)
        for slot, h in enumerate([heads[0], heads[2], heads[1], heads[3]]):
            woTh[slot * D:(slot + 1) * D, :] = wo[:, h * D:(h + 1) * D].T
        in_maps.append(
            {
                **shared,
                "wqT": np.ascontiguousarray(wqTh.astype(BF16NP)),
                "wkvT": np.ascontiguousarray(wkvTh.astype(BF16NP)),
                "woT": np.ascontiguousarray(woTh.astype(BF16NP)),
            }
        )
    return in_maps


_NC_CACHE = {}


def _get_nc():
    if "nc" not in _NC_CACHE:
        _NC_CACHE["nc"] = build_bass()
    return _NC_CACHE["nc"]


def kernel(_trace=False, **inputs):
    nc = _get_nc()
    in_maps = _host_inputs(inputs)
    res = run_bass_kernel_spmd(
        nc, in_maps, core_ids=list(range(NCORES)), trace=_trace
    )
    out = np.zeros((S, E), np.float32)
    for r in res.results:
        out += r["out_partial"]
    if _trace:
        kernel.last_exec_time_ns = res.exec_time_ns
        kernel.last_results = res
    return out.reshape(1, S, E)

